# revision 21
# baseline (speedup 1.0000x reference)
"""Sliding-window GQA attention (RoPE + attention sinks) on 8 TRN2 NeuronCores.

Problem: B=1, S=2048, H=32 q-heads, KV=8 kv-heads (GQA group 4), D=128,
sliding window 1024, causal, per-head sink logit in the softmax denominator.

Sharding: tensor-parallel over heads. Core c gets q-heads [4c, 4c+4) and kv
head c - GQA groups align exactly with cores, so there is no cross-core
communication at all. Each core computes 4 attention heads independently;
the host concatenates the 8 per-core outputs along the head axis.

Per-core kernel (all compute in bf16 with f32 PSUM accumulation):
  1. RoPE applied on device (DVE + GpSimd) in natural [s, d] layout.
  2. DMA-xbar transpose q/k to [d, s] layout for the matmuls.
  3. Key-block-outer QK^T: scoresT[k, q] in PSUM (kT block stationary,
     amortized over up to 9 query blocks).
  4. ScalarE exp(SCALE * scoresT) -> transposed probabilities pT (bf16);
     the two interleaved heads' scores are exp'd in ONE activation
     instruction (paired across PSUM bufs) to halve ACT fixed overhead.
  5. Sliding-window/causal masking applied post-exp as a 0/1 multiply on the
     two diagonal (partial) blocks of each key block.
  6. PV: out[q, d] = sum_j pT_j.T @ [v_j | 1]  - the pT chunk is the
     stationary operand (M=q=128) and v is extended with a ones column
     (N=129 <= 512 moving limit), so column 128 accumulates the softmax
     denominator for free.
  7. Normalize: denom += exp(sink); out *= 1/denom (per-partition scalar).

All DRAM I/O uses host-preswizzled [partition, linear] layouts so each DMA
descriptor covers 4KB+ contiguous per partition, and transfers are spread
over the SP / Activation / Pool DMA queues. Loop-invariant tables
(cos/sin/mask/sink) are loaded outside the timing loop.
"""

import sys

sys.path.insert(0, "/opt/trn_rl_repo")

import numpy as np
import ml_dtypes

import concourse.bass as bass
from concourse import mybir, bacc
from concourse.tile import TileContext
from concourse.bass_utils import run_bass_kernel_spmd

# ---- problem constants (hardcoded per spec) ----
B, S, H, KV, D = 1, 2048, 32, 8, 128
NCORES = 8
HPC = H // NCORES          # 4 q heads per core
WINDOW = 1024
NB = S // 128              # 16 seq blocks
WB = WINDOW // 128         # 8 window blocks
SCALE = 0.08838834764831845
ROPE_BASE = 10000.0

BF16 = mybir.dt.bfloat16
F32 = mybir.dt.float32
npbf16 = ml_dtypes.bfloat16

_CACHE = {}
PE_INTERLEAVE = True
SPLIT_PREP = True
SPLIT_NORM = True
NORM_CHUNK = 4
DEFER_TAIL_NORM = True
GROUPW = 2  # heads interleaved per group
OSTAGE_BUFS = 4
Q_ROPE_BUFS = 2
SMALL_BUFS = 2
QIO_BUFS = 3
QTP_BUFS = 4
PTP_EXTRA = 2
POOL_MODE = "stack"
PV_LAG = 2
MASK_ENGINE = "dve"   # "dve" | "gpsimd" | "alt"
NORM_ENGINE = "dve"   # "dve" | "gpsimd"
PAIRED_EXP = False
ROPE_T1_ENGINE = "gpsimd"
STAGGERED_LOOP = True


def _emit_consts(nc, tc, pools, tensors):
    """Loop-invariant tables: cos/sin/mask/sinkexp (scalar DMA queue)."""
    constp = pools[0]
    q_d, k_d, v_d, cos_d, sin_d, se_d, mask_d, out_d = tensors
    cos_sb = constp.tile([128, NB, D], BF16)
    sin_sb = constp.tile([128, NB, D], BF16)
    nc.scalar.dma_start(out=cos_sb, in_=cos_d.ap().rearrange("p (j d) -> p j d", d=D))
    nc.scalar.dma_start(out=sin_sb, in_=sin_d.ap().rearrange("p (j d) -> p j d", d=D))
    maskc = constp.tile([128, 2, 128], BF16)
    nc.scalar.dma_start(out=maskc, in_=mask_d.ap())
    se_sb = constp.tile([128, HPC], F32)
    nc.scalar.dma_start(
        out=se_sb, in_=bass.AP(tensor=se_d, offset=0, ap=[[0, 128], [1, HPC]])
    )
    return cos_sb, sin_sb, maskc, se_sb


def _emit_body(nc, tc, pools, tensors, consts):
    """Emit one full forward pass (4 heads) into the TileContext."""
    constp, qio, ropep, qtp, ptp, psc, pso, ostagep, smallp = pools
    q_d, k_d, v_d, cos_d, sin_d, se_d, mask_d, out_d = tensors
    cos_sb, sin_sb, maskc, se_sb = consts

    # ---- per-iteration input loads + rope, piece-pipelined across 3 rings ----
    # Ring issue order == issuing engine's program order, so placement is by
    # need-time: ACT carries the startup-critical pieces (its exps come right
    # after), SP carries bulk + late transposes + stores, Pool (SWDGE) bulk.
    k_src = k_d.ap().rearrange("p (j d) -> p j d", d=D)
    v_src = v_d.ap().rearrange("p (j d) -> p j d", d=D + 1)
    qsrcs = {h: q_d.ap()[h].rearrange("p (j d) -> p j d", d=D) for h in range(HPC)}

    k_nat = qio.tile([128, NB, D], BF16, tag="knat", bufs=2)
    kT = qio.tile([128, NB, D], BF16, tag="kT", bufs=2)
    v_sb = qio.tile([128, NB, D + 1], BF16, tag="vsb", bufs=2)
    qnats, qTs, ptss, ostages = {}, {}, {}, {}
    for h in range(HPC):
        qnats[h] = qio.tile([128, NB, D], BF16, tag="qnat", bufs=QIO_BUFS,
                            name=f"qnat{h}")
        qTs[h] = qtp.tile([128, NB, D], BF16, tag="qT", name=f"qT{h}")
        ptss[h] = []
        ostages[h] = ostagep.tile(
            [128, NB, D + 1], BF16, tag="ostage", name=f"ostage{h}"
        )
    t1k = ropep.tile([128, NB, D], BF16, tag="t1k", bufs=1)
    xrk = ropep.tile([128, NB, D], BF16, tag="xrk", bufs=1)
    t1qs, xrqs = {}, {}
    for h in range(HPC):
        t1qs[h] = ropep.tile([128, NB, D], BF16, tag="t1q", bufs=Q_ROPE_BUFS,
                             name=f"t1q{h}")
        xrqs[h] = ropep.tile([128, NB, D], BF16, tag="xrq", bufs=Q_ROPE_BUFS,
                             name=f"xrq{h}")

    def load(eng, dst, src, lo, hi):
        eng.dma_start(out=dst[:, lo:hi, :], in_=src[:, lo:hi, :])

    def rope_ops(h_or_k, lo, hi):
        """DVE/Pool rope math for blocks [lo,hi) (no transpose)."""
        if h_or_k == "k":
            nat, t1, xr, t1e = k_nat, t1k, xrk, nc.vector
        else:
            h = h_or_k
            nat, t1, xr = qnats[h], t1qs[h], xrqs[h]
            t1e = nc.vector if (h == 0 or ROPE_T1_ENGINE != "gpsimd") else nc.gpsimd
        sw = bass.AP(tensor=nat.tensor, offset=nat.offset + 64 + lo * D,
                     ap=[nat.ap[0], [D, hi - lo], [-64, 2], [1, 64]])
        t1e.tensor_mul(t1[:, lo:hi, :], sw, sin_sb[:, lo:hi, :])
        nc.vector.tensor_mul(xr[:, lo:hi, :], nat[:, lo:hi, :],
                             cos_sb[:, lo:hi, :])
        nc.vector.tensor_add(xr[:, lo:hi, :], xr[:, lo:hi, :],
                             t1[:, lo:hi, :])

    def xpose(eng, h_or_k, lo, hi):
        if h_or_k == "k":
            xr, xt = xrk, kT
        else:
            xr, xt = xrqs[h_or_k], qTs[h_or_k]
        eng.dma_start_transpose(out=xt[:, lo:hi, :], in_=xr[:, lo:hi, :])

    # -- load issues (per-ring program order) --
    load(nc.scalar, k_nat, k_src, 0, 3)       # ACT: critical k head
    load(nc.sync, qnats[0], qsrcs[0], 0, 5)
    load(nc.gpsimd, v_sb, v_src, 0, 8)
    load(nc.scalar, qnats[1], qsrcs[1], 0, 5)
    load(nc.sync, qnats[0], qsrcs[0], 5, 9)
    load(nc.gpsimd, qnats[2], qsrcs[2], 0, 9)
    load(nc.scalar, qnats[1], qsrcs[1], 5, 9)
    load(nc.sync, k_nat, k_src, 3, 9)
    load(nc.scalar, qnats[1], qsrcs[1], 9, 16)
    load(nc.sync, qnats[0], qsrcs[0], 9, 16)
    load(nc.sync, k_nat, k_src, 9, 16)

    # -- rope math, priority order --
    rope_ops("k", 0, 3)
    rope_ops(0, 0, 5)
    rope_ops(1, 0, 5)

    # -- early transposes on ACT (precede the exps in ACT program order) --
    xpose(nc.scalar, "k", 0, 3)
    xpose(nc.scalar, 0, 0, 5)
    xpose(nc.scalar, 1, 0, 5)

    rope_ops(0, 5, 9)
    rope_ops(1, 5, 9)
    xpose(nc.scalar, 0, 5, 9)
    xpose(nc.scalar, 1, 5, 9)

    rope_ops("k", 3, 9)
    rope_ops(0, 9, 16)
    rope_ops(1, 9, 16)
    rope_ops("k", 9, 16)

    # late loads (issue behind the early ACT work; needed mid-iteration)
    load(nc.gpsimd, v_sb, v_src, 8, 16)
    load(nc.gpsimd, qnats[2], qsrcs[2], 9, 16)
    load(nc.gpsimd, qnats[3], qsrcs[3], 0, 9)
    load(nc.gpsimd, qnats[3], qsrcs[3], 9, 16)

    # late transposes on SP (block only SP's late store issues)
    xpose(nc.sync, "k", 3, 9)
    xpose(nc.sync, 0, 9, 16)
    xpose(nc.sync, 1, 9, 16)
    xpose(nc.sync, "k", 9, 16)

    rope_ops(2, 0, 9)
    xpose(nc.sync, 2, 0, 9)
    rope_ops(2, 9, 16)
    rope_ops(3, 0, 9)
    xpose(nc.sync, 2, 9, 16)
    xpose(nc.sync, 3, 0, 9)
    rope_ops(3, 9, 16)
    xpose(nc.sync, 3, 9, 16)

    def qkt_chunk_thunks(h, j, qT, sc):
        nq = min(j + WB, NB - 1) - j + 1
        sc_flat = sc[:, :nq, :].opt()
        rhs_full = qT[:, j : j + nq, :].opt()
        thunks = []
        spans = [(c0, min(512, nq * 128 - c0)) for c0 in range(0, nq * 128, 512)]
        for c0, n in spans:
            thunks.append(
                lambda c0=c0, n=n: nc.tensor.matmul(
                    sc_flat[:, c0 : c0 + n],
                    kT[:, j, :],
                    rhs_full[:, c0 : c0 + n],
                    start=True,
                    stop=True,
                )
            )
        return thunks

    def exp_step(j, scs, pts):
        """One ACT instruction exp'ing all live heads' scores for step j."""
        nq = min(j + WB, NB - 1) - j + 1
        if PAIRED_EXP and len(scs) == 2:
            sc0, sc1 = scs
            stride = sc1.offset - sc0.offset
            src = bass.AP(
                tensor=sc0.tensor, offset=sc0.offset,
                ap=[sc0.ap[0], [stride, 2], [128, nq], [1, 128]],
            )
            pt0, pt1 = pts
            pstride = pt1.offset - pt0.offset
            dst = bass.AP(
                tensor=pt0.tensor, offset=pt0.offset,
                ap=[pt0.ap[0], [pstride, 2], [128, nq], [1, 128]],
            )
            nc.scalar.activation(
                dst, src, mybir.ActivationFunctionType.Exp, scale=SCALE
            )
        else:
            for sc, pt in zip(scs, pts):
                nc.scalar.activation(
                    pt[:, :nq, :], sc[:, :nq, :],
                    mybir.ActivationFunctionType.Exp, scale=SCALE,
                )

    def mask_step(h, j, pt):
        if MASK_ENGINE == "alt":
            eng = nc.vector if (j % 2 == 0) else nc.gpsimd
        else:
            eng = nc.vector if MASK_ENGINE == "dve" else nc.gpsimd
        if j + WB <= NB - 1:
            two = bass.AP(
                tensor=pt.tensor,
                offset=pt.offset,
                ap=[pt.ap[0], [WB * 128, 2], [1, 128]],
            )
            eng.tensor_mul(two, two, maskc)
        else:
            eng.tensor_mul(pt[:, 0, :], pt[:, 0, :], maskc[:, 0, :])

    def pv_thunks(h, i, pts, ostage):
        j0 = max(0, i - WB)
        acc = pso.tile([128, D + 1], F32, tag="acc", name="acc")
        thunks = []
        for j in range(j0, i + 1):
            thunks.append(
                lambda j=j, acc=acc: nc.tensor.matmul(
                    acc,
                    pts[j][:, i - j, :],
                    v_sb[:, j, :],
                    start=(j == j0),
                    stop=(j == i),
                )
            )
        thunks.append(lambda acc=acc: nc.vector.tensor_copy(ostage[:, i, :], acc))
        return thunks

    def pv_evac(h, i, pts, ostage):
        for t in pv_thunks(h, i, pts, ostage):
            t()

    def normalize_store(h, lo=0, hi=NB):
        ostage = ostages[h]
        nblk = hi - lo
        dview = ostage[:, lo:hi, D]  # [128, nblk] strided denominators
        dt = smallp.tile([128, NB], F32, tag="dt")
        nc.vector.tensor_scalar_add(dt[:, :nblk], dview, se_sb[:, h : h + 1])
        rt = smallp.tile([128, NB], F32, tag="rt")
        nc.vector.reciprocal(rt[:, :nblk], dt[:, :nblk])
        neng = nc.vector if NORM_ENGINE == "dve" else nc.gpsimd
        for i in range(lo, hi):
            neng.tensor_scalar_mul(
                ostage[:, i, :D], ostage[:, i, :D], rt[:, i - lo : i - lo + 1]
            )
        nc.sync.dma_start(
            out=out_d.ap()[h].rearrange("p (j d) -> p j d", d=D + 1)[:, lo:hi, :],
            in_=ostage[:, lo:hi, :],
        )

    deferred = []
    for h0 in range(0, HPC, GROUPW):
        pair = tuple(range(h0, h0 + GROUPW))
        for j in range(NB):
            scs, pts = [], []
            for h in pair:
                sc = psc.tile([128, WB + 1, 128], F32, tag="sc", name=f"sc{h}")
                pt = ptp.tile([128, WB + 1, 128], BF16, tag="pt", name=f"pt{h}")
                scs.append(sc)
                pts.append(pt)
                qk = qkt_chunk_thunks(h, j, qTs[h], sc)
                pv = (
                    pv_thunks(h, j - PV_LAG, ptss[h], ostages[h])
                    if j >= PV_LAG else []
                )
                npv, nqk, pvi = len(pv), len(qk), 0
                for qi, qt_ in enumerate(qk):
                    qt_()
                    take = ((qi + 1) * npv) // nqk - pvi
                    for _ in range(take):
                        pv[pvi](); pvi += 1
                while pvi < npv:
                    pv[pvi](); pvi += 1
                ptss[h].append(pt)
            exp_step(j, scs, pts)
            for h, pt in zip(pair, pts):
                mask_step(h, j, pt)
                if SPLIT_NORM and j >= NORM_CHUNK + PV_LAG and (
                    (j - PV_LAG) % NORM_CHUNK == 0
                ):
                    normalize_store(h, j - PV_LAG - NORM_CHUNK, j - PV_LAG)
            if j == 1 and deferred:
                for fn in deferred:
                    fn()
                deferred = []
        for i in range(NB - PV_LAG, NB):
            for h in pair:
                pv_evac(h, i, ptss[h], ostages[h])
        for h in pair:
            if SPLIT_NORM:
                done = NORM_CHUNK * ((NB - 1 - PV_LAG) // NORM_CHUNK)
                if DEFER_TAIL_NORM and h0 + GROUPW < HPC:
                    deferred.append(
                        lambda hh=h, dd=done: normalize_store(hh, dd, NB)
                    )
                else:
                    normalize_store(h, done, NB)
            else:
                normalize_store(h)
    for fn in deferred:
        fn()


def build_nc(loop_r=None, inline_inputs=None):
    """Build the per-core Bass graph. loop_r: if set, wrap the body in a
    For_i loop with that many serialized repetitions (for timing).
    inline_inputs: optional dict name->np.ndarray baked into the NEFF as
    Const tensors (timing mode: avoids per-call input upload)."""
    nc = bacc.Bacc("TRN2", target_bir_lowering=False, num_devices=NCORES)
    if inline_inputs is None:
        q_d = nc.dram_tensor("q", [HPC, 128, NB * D], BF16, kind="ExternalInput")
        k_d = nc.dram_tensor("k", [128, NB * D], BF16, kind="ExternalInput")
        v_d = nc.dram_tensor("vx", [128, NB * (D + 1)], BF16, kind="ExternalInput")
        cos_d = nc.dram_tensor("cose", [128, NB * D], BF16, kind="ExternalInput")
        sin_d = nc.dram_tensor("sine", [128, NB * D], BF16, kind="ExternalInput")
        se_d = nc.dram_tensor("sinkexp", [HPC], F32, kind="ExternalInput")
        mask_d = nc.dram_tensor("maskc", [128, 2, 128], BF16, kind="ExternalInput")
    else:
        ii = inline_inputs
        q_d = nc.inline_tensor(ii["q"], "q")
        k_d = nc.inline_tensor(ii["k"], "k")
        v_d = nc.inline_tensor(ii["vx"], "vx")
        cos_d = nc.inline_tensor(ii["cose"], "cose")
        sin_d = nc.inline_tensor(ii["sine"], "sine")
        se_d = nc.inline_tensor(ii["sinkexp"], "sinkexp")
        mask_d = nc.inline_tensor(ii["maskc"], "maskc")
    out_d = nc.dram_tensor("out", [HPC, 128, NB * (D + 1)], BF16, kind="ExternalOutput")
    tensors = (q_d, k_d, v_d, cos_d, sin_d, se_d, mask_d, out_d)

    with TileContext(nc, pool_alloc_mode=POOL_MODE) as tc:
        with (
            tc.tile_pool(name="consts", bufs=1) as constp,
            tc.tile_pool(name="qio", bufs=QIO_BUFS) as qio,
            tc.tile_pool(name="ropep", bufs=3) as ropep,
            tc.tile_pool(name="qtp", bufs=QTP_BUFS) as qtp,
            tc.tile_pool(name="ptp", bufs=GROUPW * (WB + 1 + PV_LAG) + PTP_EXTRA) as ptp,
            tc.tile_pool(name="psc", bufs=2, space="PSUM") as psc,
            tc.tile_pool(name="pso", bufs=2, space="PSUM") as pso,
            tc.tile_pool(name="ostagep", bufs=OSTAGE_BUFS) as ostagep,
            tc.tile_pool(name="smallp", bufs=SMALL_BUFS) as smallp,
        ):
            pools = (constp, qio, ropep, qtp, ptp, psc, pso, ostagep, smallp)
            consts = _emit_consts(nc, tc, pools, tensors)
            if loop_r is None:
                _emit_body(nc, tc, pools, tensors, consts)
            else:
                with tc.For_i(0, loop_r, 1, staggered_reset=STAGGERED_LOOP):
                    _emit_body(nc, tc, pools, tensors, consts)
    nc.compile()
    return nc


def _swizzle(x2d):
    """[S, Dx] -> [128, NB*Dx] with row j*128+p at [p, j*Dx:(j+1)*Dx]."""
    Dx = x2d.shape[1]
    return np.ascontiguousarray(
        x2d.reshape(NB, 128, Dx).transpose(1, 0, 2).reshape(128, NB * Dx)
    )


def _prep_in_maps(q, k, v, positions, sinks):
    pos = np.asarray(positions)[0].astype(np.float32)  # [S]
    inv_freq = 1.0 / (ROPE_BASE ** (np.arange(0, D, 2, dtype=np.float32) / D))
    ang = pos[:, None] * inv_freq[None, :]  # [S, 64]
    cos = np.cos(ang).astype(np.float32)
    sin = np.sin(ang).astype(np.float32)
    cos_ext = _swizzle(np.concatenate([cos, cos], 1).astype(npbf16))
    sin_sgn = _swizzle(np.concatenate([-sin, sin], 1).astype(npbf16))

    bidx = np.arange(128)
    mr = (bidx[:, None] <= bidx[None, :]).astype(npbf16)  # causal diag: k<=q
    ml = (bidx[:, None] > bidx[None, :]).astype(npbf16)   # window-left diag: k>q
    maskc = np.ascontiguousarray(np.stack([mr, ml], axis=1))  # [128, 2, 128]

    sinkexp = np.exp(np.asarray(sinks).astype(np.float32))  # [H]

    q0 = np.asarray(q)[0].astype(npbf16)   # [S, H, D]
    k0 = np.asarray(k)[0].astype(npbf16)   # [S, KV, D]
    v0 = np.asarray(v)[0].astype(np.float32)
    ones = np.ones((S, 1), np.float32)

    in_maps = []
    for c in range(NCORES):
        vx = np.concatenate([v0[:, c, :], ones], axis=1).astype(npbf16)
        qh = np.stack(
            [_swizzle(q0[:, HPC * c + h, :]) for h in range(HPC)], axis=0
        )
        in_maps.append(
            {
                "q": np.ascontiguousarray(qh),
                "k": _swizzle(k0[:, c, :]),
                "vx": _swizzle(vx),
                "cose": cos_ext,
                "sine": sin_sgn,
                "sinkexp": np.ascontiguousarray(sinkexp[HPC * c : HPC * (c + 1)]),
                "maskc": maskc,
            }
        )
    return in_maps


def kernel(q, k, v, positions, sinks):
    if "nc" not in _CACHE:
        _CACHE["nc"] = build_nc()
    nc = _CACHE["nc"]
    in_maps = _prep_in_maps(q, k, v, positions, sinks)
    res = run_bass_kernel_spmd(nc, in_maps, core_ids=list(range(NCORES)))
    out = np.empty((B, S, H, D), np.float32)
    for c in range(NCORES):
        r = res.results[c]["out"].astype(np.float32)  # [HPC, 128, NB*(D+1)]
        r = r.reshape(HPC, 128, NB, D + 1)[..., :D]
        r = r.transpose(0, 2, 1, 3).reshape(HPC, S, D)
        out[0, :, HPC * c : HPC * (c + 1), :] = r.transpose(1, 0, 2)
    return out


# revision 24
# speedup vs baseline: 1.1434x; 1.1434x over previous
"""Sliding-window GQA attention (RoPE + attention sinks) on 8 TRN2 NeuronCores.

Problem: B=1, S=2048, H=32 q-heads, KV=8 kv-heads (GQA group 4), D=128,
sliding window 1024, causal, per-head sink logit in the softmax denominator.

Sharding: tensor-parallel over heads. Core c gets q-heads [4c, 4c+4) and kv
head c - GQA groups align exactly with cores, so there is no cross-core
communication at all. Each core computes 4 attention heads independently;
the host concatenates the 8 per-core outputs along the head axis.

Per-core kernel (all compute in bf16 with f32 PSUM accumulation):
  1. RoPE applied on device (DVE + GpSimd) in natural [s, d] layout.
  2. DMA-xbar transpose q/k to [d, s] layout for the matmuls.
  3. Key-block-outer QK^T: scoresT[k, q] in PSUM (kT block stationary,
     amortized over up to 9 query blocks).
  4. ScalarE exp(SCALE * scoresT) -> transposed probabilities pT (bf16);
     the two interleaved heads' scores are exp'd in ONE activation
     instruction (paired across PSUM bufs) to halve ACT fixed overhead.
  5. Sliding-window/causal masking applied post-exp as a 0/1 multiply on the
     two diagonal (partial) blocks of each key block.
  6. PV: out[q, d] = sum_j pT_j.T @ [v_j | 1]  - the pT chunk is the
     stationary operand (M=q=128) and v is extended with a ones column
     (N=129 <= 512 moving limit), so column 128 accumulates the softmax
     denominator for free.
  7. Normalize: denom += exp(sink); out *= 1/denom (per-partition scalar).

All DRAM I/O uses host-preswizzled [partition, linear] layouts so each DMA
descriptor covers 4KB+ contiguous per partition, and transfers are spread
over the SP / Activation / Pool DMA queues. Loop-invariant tables
(cos/sin/mask/sink) are loaded outside the timing loop.
"""

import sys

sys.path.insert(0, "/opt/trn_rl_repo")

import numpy as np
import ml_dtypes

import concourse.bass as bass
from concourse import mybir, bacc
from concourse.tile import TileContext
from concourse.bass_utils import run_bass_kernel_spmd

# ---- problem constants (hardcoded per spec) ----
B, S, H, KV, D = 1, 2048, 32, 8, 128
NCORES = 8
HPC = H // NCORES          # 4 q heads per core
WINDOW = 1024
NB = S // 128              # 16 seq blocks
WB = WINDOW // 128         # 8 window blocks
SCALE = 0.08838834764831845
ROPE_BASE = 10000.0

BF16 = mybir.dt.bfloat16
F32 = mybir.dt.float32
npbf16 = ml_dtypes.bfloat16

_CACHE = {}
PE_INTERLEAVE = True
SPLIT_PREP = True
SPLIT_NORM = True
NORM_CHUNK = 4
DEFER_TAIL_NORM = True
GROUPW = 2  # heads interleaved per group
OSTAGE_BUFS = 4
Q_ROPE_BUFS = 2
SMALL_BUFS = 2
QIO_BUFS = 3
QTP_BUFS = 4
PTP_EXTRA = 2
POOL_MODE = "stack"
PV_LAG = 2
MASK_ENGINE = "dve"   # "dve" | "gpsimd" | "alt"
NORM_ENGINE = "dve"   # "dve" | "gpsimd"
PAIRED_EXP = False
ROPE_T1_ENGINE = "gpsimd"
STAGGERED_LOOP = True


def _emit_consts(nc, tc, pools, tensors):
    """Loop-invariant tables: cos/sin/mask/sinkexp (scalar DMA queue)."""
    constp = pools[0]
    q_d, k_d, v_d, cos_d, sin_d, se_d, mask_d, out_d = tensors
    cos_sb = constp.tile([128, NB, D], BF16)
    sin_sb = constp.tile([128, NB, D], BF16)
    nc.scalar.dma_start(out=cos_sb, in_=cos_d.ap().rearrange("p (j d) -> p j d", d=D))
    nc.scalar.dma_start(out=sin_sb, in_=sin_d.ap().rearrange("p (j d) -> p j d", d=D))
    maskc = constp.tile([128, 2, 128], BF16)
    nc.scalar.dma_start(out=maskc, in_=mask_d.ap())
    se_sb = constp.tile([128, HPC], F32)
    nc.scalar.dma_start(
        out=se_sb, in_=bass.AP(tensor=se_d, offset=0, ap=[[0, 128], [1, HPC]])
    )
    return cos_sb, sin_sb, maskc, se_sb


def _emit_body(nc, tc, pools, tensors, consts):
    """Emit one full forward pass (4 heads) into the TileContext."""
    constp, qio, ropep, qtp, ptp, psc, pso, ostagep, smallp = pools
    q_d, k_d, v_d, cos_d, sin_d, se_d, mask_d, out_d = tensors
    cos_sb, sin_sb, maskc, se_sb = consts

    # ---- per-iteration input loads + rope, piece-pipelined across 3 rings ----
    # Ring issue order == issuing engine's program order, so placement is by
    # need-time: ACT carries the startup-critical pieces (its exps come right
    # after), SP carries bulk + late transposes + stores, Pool (SWDGE) bulk.
    k_src = k_d.ap().rearrange("p (j d) -> p j d", d=D)
    v_src = v_d.ap().rearrange("p (j d) -> p j d", d=D + 1)
    qsrcs = {h: q_d.ap()[h].rearrange("p (j d) -> p j d", d=D) for h in range(HPC)}

    k_nat = qio.tile([128, NB, D], BF16, tag="knat", bufs=2)
    kT = qio.tile([128, NB, D], BF16, tag="kT", bufs=2)
    v_sb = qio.tile([128, NB, D + 1], BF16, tag="vsb", bufs=2)
    qnats, qTs, ptss, ostages = {}, {}, {}, {}
    for h in range(HPC):
        qnats[h] = qio.tile([128, NB, D], BF16, tag="qnat", bufs=QIO_BUFS,
                            name=f"qnat{h}")
        qTs[h] = qtp.tile([128, NB, D], BF16, tag="qT", name=f"qT{h}")
        ptss[h] = []
        ostages[h] = ostagep.tile(
            [128, NB, D + 1], BF16, tag="ostage", name=f"ostage{h}"
        )
    t1k = ropep.tile([128, NB, D], BF16, tag="t1k", bufs=1)
    xrk = ropep.tile([128, NB, D], BF16, tag="xrk", bufs=1)
    t1qs, xrqs = {}, {}
    for h in range(HPC):
        t1qs[h] = ropep.tile([128, NB, D], BF16, tag="t1q", bufs=Q_ROPE_BUFS,
                             name=f"t1q{h}")
        xrqs[h] = ropep.tile([128, NB, D], BF16, tag="xrq", bufs=Q_ROPE_BUFS,
                             name=f"xrq{h}")

    def load(eng, dst, src, lo, hi):
        eng.dma_start(out=dst[:, lo:hi, :], in_=src[:, lo:hi, :])

    def rope_ops(h_or_k, lo, hi):
        """DVE/Pool rope math for blocks [lo,hi) (no transpose)."""
        if h_or_k == "k":
            nat, t1, xr, t1e = k_nat, t1k, xrk, nc.vector
        else:
            h = h_or_k
            nat, t1, xr = qnats[h], t1qs[h], xrqs[h]
            t1e = nc.vector if (h == 0 or ROPE_T1_ENGINE != "gpsimd") else nc.gpsimd
        sw = bass.AP(tensor=nat.tensor, offset=nat.offset + 64 + lo * D,
                     ap=[nat.ap[0], [D, hi - lo], [-64, 2], [1, 64]])
        t1e.tensor_mul(t1[:, lo:hi, :], sw, sin_sb[:, lo:hi, :])
        nc.vector.tensor_mul(xr[:, lo:hi, :], nat[:, lo:hi, :],
                             cos_sb[:, lo:hi, :])
        nc.vector.tensor_add(xr[:, lo:hi, :], xr[:, lo:hi, :],
                             t1[:, lo:hi, :])

    def xpose(eng, h_or_k, lo, hi):
        if h_or_k == "k":
            xr, xt = xrk, kT
        else:
            xr, xt = xrqs[h_or_k], qTs[h_or_k]
        eng.dma_start_transpose(out=xt[:, lo:hi, :], in_=xr[:, lo:hi, :])

    # -- load issues (per-ring program order) --
    load(nc.scalar, k_nat, k_src, 0, 3)       # ACT: critical k head
    load(nc.sync, qnats[0], qsrcs[0], 0, 5)
    load(nc.gpsimd, v_sb, v_src, 0, 8)
    load(nc.scalar, qnats[1], qsrcs[1], 0, 5)
    load(nc.sync, qnats[0], qsrcs[0], 5, 9)
    load(nc.gpsimd, qnats[2], qsrcs[2], 0, 9)
    load(nc.scalar, qnats[1], qsrcs[1], 5, 9)
    load(nc.sync, k_nat, k_src, 3, 9)
    load(nc.scalar, qnats[1], qsrcs[1], 9, 16)
    load(nc.sync, qnats[0], qsrcs[0], 9, 16)
    load(nc.sync, k_nat, k_src, 9, 16)

    # -- rope math, priority order --
    rope_ops("k", 0, 3)
    rope_ops(0, 0, 5)
    rope_ops(1, 0, 5)

    # -- early transposes on ACT (precede the exps in ACT program order) --
    xpose(nc.scalar, "k", 0, 3)
    xpose(nc.scalar, 0, 0, 5)
    xpose(nc.scalar, 1, 0, 5)

    rope_ops(0, 5, 9)
    rope_ops(1, 5, 9)
    xpose(nc.scalar, 0, 5, 9)
    xpose(nc.scalar, 1, 5, 9)

    rope_ops("k", 3, 9)
    rope_ops(0, 9, 16)
    rope_ops(1, 9, 16)
    rope_ops("k", 9, 16)

    # late loads (issue behind the early ACT work; needed mid-iteration)
    load(nc.gpsimd, v_sb, v_src, 8, 16)
    load(nc.gpsimd, qnats[2], qsrcs[2], 9, 16)
    load(nc.gpsimd, qnats[3], qsrcs[3], 0, 9)
    load(nc.gpsimd, qnats[3], qsrcs[3], 9, 16)

    # late transposes on SP (block only SP's late store issues)
    xpose(nc.sync, "k", 3, 9)
    xpose(nc.sync, 0, 9, 16)
    xpose(nc.sync, 1, 9, 16)
    xpose(nc.sync, "k", 9, 16)

    rope_ops(2, 0, 9)
    xpose(nc.sync, 2, 0, 9)
    rope_ops(2, 9, 16)
    rope_ops(3, 0, 9)
    xpose(nc.sync, 2, 9, 16)
    xpose(nc.sync, 3, 0, 9)
    rope_ops(3, 9, 16)
    xpose(nc.sync, 3, 9, 16)

    def qkt_chunk_thunks(h, j, qT, sc):
        nq = min(j + WB, NB - 1) - j + 1
        sc_flat = sc[:, :nq, :].opt()
        rhs_full = qT[:, j : j + nq, :].opt()
        thunks = []
        spans = [(c0, min(512, nq * 128 - c0)) for c0 in range(0, nq * 128, 512)]
        for c0, n in spans:
            thunks.append(
                lambda c0=c0, n=n: nc.tensor.matmul(
                    sc_flat[:, c0 : c0 + n],
                    kT[:, j, :],
                    rhs_full[:, c0 : c0 + n],
                    start=True,
                    stop=True,
                )
            )
        return thunks

    def exp_step(j, scs, pts):
        """One ACT instruction exp'ing all live heads' scores for step j."""
        nq = min(j + WB, NB - 1) - j + 1
        if PAIRED_EXP and len(scs) == 2:
            sc0, sc1 = scs
            stride = sc1.offset - sc0.offset
            src = bass.AP(
                tensor=sc0.tensor, offset=sc0.offset,
                ap=[sc0.ap[0], [stride, 2], [128, nq], [1, 128]],
            )
            pt0, pt1 = pts
            pstride = pt1.offset - pt0.offset
            dst = bass.AP(
                tensor=pt0.tensor, offset=pt0.offset,
                ap=[pt0.ap[0], [pstride, 2], [128, nq], [1, 128]],
            )
            nc.scalar.activation(
                dst, src, mybir.ActivationFunctionType.Exp, scale=SCALE
            )
        else:
            for sc, pt in zip(scs, pts):
                nc.scalar.activation(
                    pt[:, :nq, :], sc[:, :nq, :],
                    mybir.ActivationFunctionType.Exp, scale=SCALE,
                )

    def mask_step(h, j, pt):
        if MASK_ENGINE == "alt":
            eng = nc.vector if (j % 2 == 0) else nc.gpsimd
        else:
            eng = nc.vector if MASK_ENGINE == "dve" else nc.gpsimd
        if j + WB <= NB - 1:
            two = bass.AP(
                tensor=pt.tensor,
                offset=pt.offset,
                ap=[pt.ap[0], [WB * 128, 2], [1, 128]],
            )
            eng.tensor_mul(two, two, maskc)
        else:
            eng.tensor_mul(pt[:, 0, :], pt[:, 0, :], maskc[:, 0, :])

    def pv_thunks(h, i, pts, ostage):
        j0 = max(0, i - WB)
        acc = pso.tile([128, D + 1], F32, tag="acc", name="acc")
        thunks = []
        for j in range(j0, i + 1):
            thunks.append(
                lambda j=j, acc=acc: nc.tensor.matmul(
                    acc,
                    pts[j][:, i - j, :],
                    v_sb[:, j, :],
                    start=(j == j0),
                    stop=(j == i),
                )
            )
        thunks.append(lambda acc=acc: nc.vector.tensor_copy(ostage[:, i, :], acc))
        return thunks

    def pv_evac(h, i, pts, ostage):
        for t in pv_thunks(h, i, pts, ostage):
            t()

    def normalize_store(h, lo=0, hi=NB):
        ostage = ostages[h]
        nblk = hi - lo
        dview = ostage[:, lo:hi, D]  # [128, nblk] strided denominators
        dt = smallp.tile([128, NB], F32, tag="dt")
        nc.vector.tensor_scalar_add(dt[:, :nblk], dview, se_sb[:, h : h + 1])
        rt = smallp.tile([128, NB], F32, tag="rt")
        nc.vector.reciprocal(rt[:, :nblk], dt[:, :nblk])
        neng = nc.vector if NORM_ENGINE == "dve" else nc.gpsimd
        for i in range(lo, hi):
            neng.tensor_scalar_mul(
                ostage[:, i, :D], ostage[:, i, :D], rt[:, i - lo : i - lo + 1]
            )
        nc.sync.dma_start(
            out=out_d.ap()[h].rearrange("p (j d) -> p j d", d=D + 1)[:, lo:hi, :],
            in_=ostage[:, lo:hi, :],
        )

    deferred = []
    for h0 in range(0, HPC, GROUPW):
        pair = tuple(range(h0, h0 + GROUPW))
        for j in range(NB):
            scs, pts = [], []
            for h in pair:
                sc = psc.tile([128, WB + 1, 128], F32, tag="sc", name=f"sc{h}")
                pt = ptp.tile([128, WB + 1, 128], BF16, tag="pt", name=f"pt{h}")
                scs.append(sc)
                pts.append(pt)
                qk = qkt_chunk_thunks(h, j, qTs[h], sc)
                pv = (
                    pv_thunks(h, j - PV_LAG, ptss[h], ostages[h])
                    if j >= PV_LAG else []
                )
                npv, nqk, pvi = len(pv), len(qk), 0
                for qi, qt_ in enumerate(qk):
                    qt_()
                    take = ((qi + 1) * npv) // nqk - pvi
                    for _ in range(take):
                        pv[pvi](); pvi += 1
                while pvi < npv:
                    pv[pvi](); pvi += 1
                ptss[h].append(pt)
            exp_step(j, scs, pts)
            for h, pt in zip(pair, pts):
                mask_step(h, j, pt)
                if SPLIT_NORM and j >= NORM_CHUNK + PV_LAG and (
                    (j - PV_LAG) % NORM_CHUNK == 0
                ):
                    normalize_store(h, j - PV_LAG - NORM_CHUNK, j - PV_LAG)
            if j == 1 and deferred:
                for fn in deferred:
                    fn()
                deferred = []
        for i in range(NB - PV_LAG, NB):
            for h in pair:
                pv_evac(h, i, ptss[h], ostages[h])
        for h in pair:
            if SPLIT_NORM:
                done = NORM_CHUNK * ((NB - 1 - PV_LAG) // NORM_CHUNK)
                if DEFER_TAIL_NORM and h0 + GROUPW < HPC:
                    deferred.append(
                        lambda hh=h, dd=done: normalize_store(hh, dd, NB)
                    )
                else:
                    normalize_store(h, done, NB)
            else:
                normalize_store(h)
    for fn in deferred:
        fn()


def build_nc(loop_r=None, inline_inputs=None):
    """Build the per-core Bass graph. loop_r: if set, wrap the body in a
    For_i loop with that many serialized repetitions (for timing).
    inline_inputs: optional dict name->np.ndarray baked into the NEFF as
    Const tensors (timing mode: avoids per-call input upload)."""
    nc = bacc.Bacc("TRN2", target_bir_lowering=False, num_devices=NCORES)
    if inline_inputs is None:
        q_d = nc.dram_tensor("q", [HPC, 128, NB * D], BF16, kind="ExternalInput")
        k_d = nc.dram_tensor("k", [128, NB * D], BF16, kind="ExternalInput")
        v_d = nc.dram_tensor("vx", [128, NB * (D + 1)], BF16, kind="ExternalInput")
        cos_d = nc.dram_tensor("cose", [128, NB * D], BF16, kind="ExternalInput")
        sin_d = nc.dram_tensor("sine", [128, NB * D], BF16, kind="ExternalInput")
        se_d = nc.dram_tensor("sinkexp", [HPC], F32, kind="ExternalInput")
        mask_d = nc.dram_tensor("maskc", [128, 2, 128], BF16, kind="ExternalInput")
    else:
        ii = inline_inputs
        q_d = nc.inline_tensor(ii["q"], "q")
        k_d = nc.inline_tensor(ii["k"], "k")
        v_d = nc.inline_tensor(ii["vx"], "vx")
        cos_d = nc.inline_tensor(ii["cose"], "cose")
        sin_d = nc.inline_tensor(ii["sine"], "sine")
        se_d = nc.inline_tensor(ii["sinkexp"], "sinkexp")
        mask_d = nc.inline_tensor(ii["maskc"], "maskc")
    out_d = nc.dram_tensor("out", [HPC, 128, NB * (D + 1)], BF16, kind="ExternalOutput")
    tensors = (q_d, k_d, v_d, cos_d, sin_d, se_d, mask_d, out_d)

    with TileContext(nc, pool_alloc_mode=POOL_MODE) as tc:
        with (
            tc.tile_pool(name="consts", bufs=1) as constp,
            tc.tile_pool(name="qio", bufs=QIO_BUFS) as qio,
            tc.tile_pool(name="ropep", bufs=3) as ropep,
            tc.tile_pool(name="qtp", bufs=QTP_BUFS) as qtp,
            tc.tile_pool(name="ptp", bufs=GROUPW * (WB + 1 + PV_LAG) + PTP_EXTRA) as ptp,
            tc.tile_pool(name="psc", bufs=2, space="PSUM") as psc,
            tc.tile_pool(name="pso", bufs=2, space="PSUM") as pso,
            tc.tile_pool(name="ostagep", bufs=OSTAGE_BUFS) as ostagep,
            tc.tile_pool(name="smallp", bufs=SMALL_BUFS) as smallp,
        ):
            pools = (constp, qio, ropep, qtp, ptp, psc, pso, ostagep, smallp)
            consts = _emit_consts(nc, tc, pools, tensors)
            if loop_r is None:
                _emit_body(nc, tc, pools, tensors, consts)
            else:
                with tc.For_i(0, loop_r, 1, staggered_reset=STAGGERED_LOOP):
                    _emit_body(nc, tc, pools, tensors, consts)
    nc.compile()
    return nc


def _swizzle(x2d):
    """[S, Dx] -> [128, NB*Dx] with row j*128+p at [p, j*Dx:(j+1)*Dx]."""
    Dx = x2d.shape[1]
    return np.ascontiguousarray(
        x2d.reshape(NB, 128, Dx).transpose(1, 0, 2).reshape(128, NB * Dx)
    )


def _prep_in_maps(q, k, v, positions, sinks):
    pos = np.asarray(positions)[0].astype(np.float32)  # [S]
    inv_freq = 1.0 / (ROPE_BASE ** (np.arange(0, D, 2, dtype=np.float32) / D))
    ang = pos[:, None] * inv_freq[None, :]  # [S, 64]
    cos = np.cos(ang).astype(np.float32)
    sin = np.sin(ang).astype(np.float32)
    cos_ext = _swizzle(np.concatenate([cos, cos], 1).astype(npbf16))
    sin_sgn = _swizzle(np.concatenate([-sin, sin], 1).astype(npbf16))

    bidx = np.arange(128)
    mr = (bidx[:, None] <= bidx[None, :]).astype(npbf16)  # causal diag: k<=q
    ml = (bidx[:, None] > bidx[None, :]).astype(npbf16)   # window-left diag: k>q
    maskc = np.ascontiguousarray(np.stack([mr, ml], axis=1))  # [128, 2, 128]

    sinkexp = np.exp(np.asarray(sinks).astype(np.float32))  # [H]

    q0 = np.asarray(q)[0].astype(npbf16)   # [S, H, D]
    k0 = np.asarray(k)[0].astype(npbf16)   # [S, KV, D]
    v0 = np.asarray(v)[0].astype(np.float32)
    ones = np.ones((S, 1), np.float32)

    in_maps = []
    for c in range(NCORES):
        vx = np.concatenate([v0[:, c, :], ones], axis=1).astype(npbf16)
        qh = np.stack(
            [_swizzle(q0[:, HPC * c + h, :]) for h in range(HPC)], axis=0
        )
        in_maps.append(
            {
                "q": np.ascontiguousarray(qh),
                "k": _swizzle(k0[:, c, :]),
                "vx": _swizzle(vx),
                "cose": cos_ext,
                "sine": sin_sgn,
                "sinkexp": np.ascontiguousarray(sinkexp[HPC * c : HPC * (c + 1)]),
                "maskc": maskc,
            }
        )
    return in_maps


def kernel(q, k, v, positions, sinks):
    if "nc" not in _CACHE:
        _CACHE["nc"] = build_nc()
    nc = _CACHE["nc"]
    in_maps = _prep_in_maps(q, k, v, positions, sinks)
    res = run_bass_kernel_spmd(nc, in_maps, core_ids=list(range(NCORES)))
    out = np.empty((B, S, H, D), np.float32)
    for c in range(NCORES):
        r = res.results[c]["out"].astype(np.float32)  # [HPC, 128, NB*(D+1)]
        r = r.reshape(HPC, 128, NB, D + 1)[..., :D]
        r = r.transpose(0, 2, 1, 3).reshape(HPC, S, D)
        out[0, :, HPC * c : HPC * (c + 1), :] = r.transpose(1, 0, 2)
    return out
